# revision 1
# baseline (speedup 1.0000x reference)
"""Low-rank (CPD) 3D conv kernel for Trainium2, SPMD across 8 NeuronCores.

Math (per reference):
  y[r,h,w,d]  = sum_c U_c_in[c,r] * x[c,h,w,d]
  y           = conv_h(conv_w(conv_d-separable 3-tap, per-rank taps U_k*))
  out[c,...]  = sum_r U_c_out[r,c] * z[r,...] + bias[c]

Distribution: data-parallel split of H (64) into 8 slabs of 8 planes; each
core reads its slab plus one halo plane on each side (zero at global edges)
and computes its output slab independently. No collectives.

Per-core pipeline (streamed over the 8 output planes):
  - mm1 with conv_h folded: 3 weight matrices W_k = U_c_in * U_kh[k] (host
    precomputed, bf16); PSUM accumulation over 2 c-tiles x 3 h-taps.
  - PSUM drain on ScalarE, casting to bf16 and de-interleaving d into
    (even,odd) halves per w-line so the d-shifts below stay 4B-aligned.
  - conv_w on VectorE: per-partition scale (tensor_scalar) + 2 fused
    scale-add passes (scalar_tensor_tensor) with +-1 w-line shifts.
  - conv_d on VectorE: same, operating across the even/odd halves.
  - mm2: lhsT = U_c_out (bf16), accumulate 2 r-tiles.
  - PSUM drain on ScalarE with per-partition bias add, re-interleaving d,
    f32 output.
"""

import numpy as np
import ml_dtypes

BF16 = ml_dtypes.bfloat16

# Problem constants (hardcoded per contest contract)
C = 256   # input channels
R = 256   # rank
CO = 256  # output channels
S = 64    # spatial extent (cube)
NCORES = 8
HP = S // NCORES          # output planes per core (8)
HS = HP + 2               # slab planes incl. halo (10)
PLANE = S * S             # 4096 elements per (w,d) plane
NCH = PLANE // 512        # 512-column matmul chunks per plane (8)

_cache = {}


def _build_program(hp=HP, wl=S):
    """Build and compile the per-core Bass program (identical on all cores).

    hp: output planes per core; wl: w-lines per plane (64 in production).
    """
    import concourse.bass as bass
    import concourse.mybir as mybir
    import concourse.tile as tile
    from concourse import bacc

    HS, PLANE, NCH = hp + 2, wl * 64, (wl * 64) // 512
    HP_ = hp

    fp32 = mybir.dt.float32
    bf16 = mybir.dt.bfloat16

    nc = bacc.Bacc("TRN2", target_bir_lowering=False, debug=False,
                   num_devices=NCORES)

    # DRAM tensors (names are the in_map keys)
    x_d = nc.dram_tensor("xs", [2, 128, HS, PLANE], bf16, kind="ExternalInput").ap()
    wkh_d = nc.dram_tensor("wkh", [3, 2, 2, 128, 128], bf16, kind="ExternalInput").ap()
    uco_d = nc.dram_tensor("uco", [2, 2, 128, 128], bf16, kind="ExternalInput").ap()
    ukw_d = nc.dram_tensor("ukw", [2, 128, 3], fp32, kind="ExternalInput").ap()
    ukd_d = nc.dram_tensor("ukd", [2, 128, 3], fp32, kind="ExternalInput").ap()
    bias_d = nc.dram_tensor("bias_t", [2, 128, 1], fp32, kind="ExternalInput").ap()
    out_d = nc.dram_tensor("out", [2, 128, HP_, PLANE], fp32, kind="ExternalOutput").ap()

    mult = mybir.AluOpType.mult
    add = mybir.AluOpType.add
    ident = mybir.ActivationFunctionType.Identity

    with tile.TileContext(nc) as tc:
        consts = tc.alloc_tile_pool(name="consts", bufs=1)
        xpool = tc.alloc_tile_pool(name="x", bufs=8)
        ypool = tc.alloc_tile_pool(name="y", bufs=3)
        tpool = tc.alloc_tile_pool(name="tmp", bufs=4)
        zpool = tc.alloc_tile_pool(name="z", bufs=2)
        zdpool = tc.alloc_tile_pool(name="zd", bufs=2)
        opool = tc.alloc_tile_pool(name="osb", bufs=2)
        ps1 = tc.alloc_tile_pool(name="ps1", bufs=2, space="PSUM")
        ps2 = tc.alloc_tile_pool(name="ps2", bufs=2, space="PSUM")

        # ---- constants ----
        wkh = [[[consts.tile([128, 128], bf16, name=f"wkh{k}{ct}{rt}", tag=f"wkh{k}{ct}{rt}")
                 for rt in range(2)] for ct in range(2)] for k in range(3)]
        for k in range(3):
            for ct in range(2):
                for rt in range(2):
                    nc.sync.dma_start(out=wkh[k][ct][rt], in_=wkh_d[k, ct, rt])
        uco = [[consts.tile([128, 128], bf16, name=f"uco{rt}{co}", tag=f"uco{rt}{co}")
                for co in range(2)] for rt in range(2)]
        for rt in range(2):
            for co in range(2):
                nc.sync.dma_start(out=uco[rt][co], in_=uco_d[rt, co])
        ukw = [consts.tile([128, 3], fp32, name=f"ukw{rt}", tag=f"ukw{rt}") for rt in range(2)]
        ukd = [consts.tile([128, 3], fp32, name=f"ukd{rt}", tag=f"ukd{rt}") for rt in range(2)]
        bia = [consts.tile([128, 1], fp32, name=f"bias{co}", tag=f"bias{co}") for co in range(2)]
        for rt in range(2):
            nc.sync.dma_start(out=ukw[rt], in_=ukw_d[rt])
            nc.sync.dma_start(out=ukd[rt], in_=ukd_d[rt])
        for co in range(2):
            nc.sync.dma_start(out=bia[co], in_=bias_d[co])

        # ---- x plane streaming ----
        xt = {}

        def get_x(p, ct):
            if (p, ct) not in xt:
                t = xpool.tile([128, PLANE], bf16, name="xplane", tag="xplane")
                nc.sync.dma_start(out=t, in_=x_d[ct, :, p, :])
                xt[(p, ct)] = t
            return xt[(p, ct)]

        NQ = PLANE // 1024  # 1024-wide psum tiles per plane

        for h in range(HP_):
            y = []
            t0s = []
            for rt in range(2):
                # --- mm1 + conv_h fold (PSUM 1024-tiles, 512 matmul halves) ---
                ysb = ypool.tile([128, PLANE], bf16, name="ysb", tag="y")
                t0 = tpool.tile([128, PLANE], bf16, name="t0t", tag="tmp")
                for q in range(NQ):
                    pt = ps1.tile([128, 1024], fp32, name="pt1", tag="ps1")
                    for half in range(2):
                        first = True
                        for k in range(3):
                            for ct in range(2):
                                nc.tensor.matmul(
                                    pt[:, half * 512:(half + 1) * 512],
                                    wkh[k][ct][rt],
                                    get_x(h + k, ct)[:, q * 1024 + half * 512:
                                                     q * 1024 + (half + 1) * 512],
                                    start=first,
                                    stop=(k == 2 and ct == 1),
                                )
                                first = False
                    # drains: f32 PSUM -> bf16 SBUF, de-interleave d.
                    # plain y (ACT) + U_kw[0]-scaled t0 (ACT)
                    src = pt.rearrange("p (w j s) -> p w s j", j=32, s=2)
                    dst = ysb.rearrange("p (w s j) -> p w s j", s=2, j=32)[
                        :, q * 16:(q + 1) * 16]
                    nc.scalar.copy(dst, src)
                    dst0 = t0.rearrange("p (w s j) -> p w s j", s=2, j=32)[
                        :, q * 16:(q + 1) * 16]
                    nc.scalar.mul(dst0, src, ukw[rt][:, 0:1])
                y.append(ysb)
                t0s.append(t0)

            # --- conv_w (VectorE + tmp from ACT drains) ---
            z = []
            for rt in range(2):
                zt = zpool.tile([128, PLANE], bf16, name="zw", tag="z")
                # z = U1*y
                nc.vector.tensor_scalar_mul(zt, y[rt], ukw[rt][:, 1:2])
                zv = zt.rearrange("p (w q) -> p w q", q=64)
                t0v = t0s[rt].rearrange("p (w q) -> p w q", q=64)
                yv = y[rt].rearrange("p (w q) -> p w q", q=64)
                # z[w] += t0[w-1]
                nc.vector.tensor_tensor(zv[:, 1:, :], t0v[:, :-1, :], zv[:, 1:, :], add)
                # t2 = U2*y ; z[w] += t2[w+1]
                t2 = tpool.tile([128, PLANE], bf16, name="t2t", tag="tmp")
                nc.vector.tensor_scalar_mul(t2, y[rt], ukw[rt][:, 2:3])
                t2v = t2.rearrange("p (w q) -> p w q", q=64)
                nc.vector.tensor_tensor(zv[:, :-1, :], t2v[:, 1:, :], zv[:, :-1, :], add)
                z.append(zt)

            # --- conv_d (VectorE scales, adds split DVE/GpSimd) ---
            zd = []
            for rt in range(2):
                zt = zdpool.tile([128, PLANE], bf16, name="zdt", tag="zd")
                a0 = tpool.tile([128, PLANE], bf16, name="a0t", tag="tmp")
                a2 = tpool.tile([128, PLANE], bf16, name="a2t", tag="tmp")
                nc.vector.tensor_scalar_mul(zt, z[rt], ukd[rt][:, 1:2])
                nc.vector.tensor_scalar_mul(a0, z[rt], ukd[rt][:, 0:1])
                nc.vector.tensor_scalar_mul(a2, z[rt], ukd[rt][:, 2:3])
                zv = zt.rearrange("p (w s j) -> p w s j", s=2, j=32)
                a0v = a0.rearrange("p (w s j) -> p w s j", s=2, j=32)
                a2v = a2.rearrange("p (w s j) -> p w s j", s=2, j=32)
                eng = nc.vector if rt == 0 else nc.gpsimd
                # even outputs d=2j:  += a0[2j-1] (j>=1), += a2[2j+1]
                eng.tensor_tensor(zv[:, :, 0, 1:], a0v[:, :, 1, :-1], zv[:, :, 0, 1:], add)
                eng.tensor_tensor(zv[:, :, 0, :], a2v[:, :, 1, :], zv[:, :, 0, :], add)
                # odd outputs d=2j+1: += a0[2j], += a2[2j+2] (j<=30)
                eng.tensor_tensor(zv[:, :, 1, :], a0v[:, :, 0, :], zv[:, :, 1, :], add)
                eng.tensor_tensor(zv[:, :, 1, :-1], a2v[:, :, 0, 1:], zv[:, :, 1, :-1], add)
                zd.append(zt)

            # --- mm2 + bias drain ---
            for co in range(2):
                osb = opool.tile([128, PLANE], fp32, name="osb", tag="osb")
                for q in range(NQ):
                    pt = ps2.tile([128, 1024], fp32, name="pt2", tag="ps2")
                    for half in range(2):
                        for rt in range(2):
                            nc.tensor.matmul(
                                pt[:, half * 512:(half + 1) * 512],
                                uco[rt][co],
                                zd[rt][:, q * 1024 + half * 512:
                                       q * 1024 + (half + 1) * 512],
                                start=(rt == 0),
                                stop=(rt == 1),
                            )
                    # drain with bias, re-interleave d
                    dst = osb.rearrange("p (w j s) -> p w s j", j=32, s=2)[
                        :, q * 16:(q + 1) * 16]
                    src = pt.rearrange("p (w s j) -> p w s j", s=2, j=32)
                    nc.scalar.activation(dst, src, ident, bias=bia[co][:, 0:1])
                nc.sync.dma_start(out=out_d[co, :, h, :], in_=osb)

        for pool in (ps2, ps1, opool, zdpool, zpool, tpool, ypool, xpool, consts):
            pool.release()

    nc.compile()
    return nc


def _host_prep(x, U_kh, U_kw, U_kd, U_c_in, U_c_out, bias):
    """Build per-core input maps (numpy only)."""
    x = np.asarray(x)
    U_kh = np.asarray(U_kh, np.float32)
    U_kw = np.asarray(U_kw, np.float32)
    U_kd = np.asarray(U_kd, np.float32)
    U_c_in = np.asarray(U_c_in, np.float32)
    U_c_out = np.asarray(U_c_out, np.float32)
    bias = np.asarray(bias, np.float32)

    xb = np.ascontiguousarray(x[0]).astype(BF16)          # [C, S, S, S]
    xb = xb.reshape(C, S, PLANE)

    # W_k[c, r] = U_c_in[c,r] * U_kh[k,r]  -> [3, ct, rt, 128, 128]
    wkh = np.empty((3, 2, 2, 128, 128), BF16)
    for k in range(3):
        wk = (U_c_in * U_kh[k][None, :]).astype(BF16)     # [C, R]
        wkh[k] = wk.reshape(2, 128, 2, 128).transpose(0, 2, 1, 3)

    uco = U_c_out.astype(BF16).reshape(2, 128, 2, 128).transpose(0, 2, 1, 3)
    uco = np.ascontiguousarray(uco)
    ukw = np.ascontiguousarray(U_kw.T.reshape(2, 128, 3))
    ukd = np.ascontiguousarray(U_kd.T.reshape(2, 128, 3))
    bias_t = np.ascontiguousarray(bias.reshape(2, 128, 1))

    in_maps = []
    for c in range(NCORES):
        slab = np.zeros((C, HS, PLANE), BF16)
        lo, hi = c * HP - 1, c * HP + HP + 1
        s0, s1 = max(lo, 0), min(hi, S)
        slab[:, s0 - lo:HS - (hi - s1)] = xb[:, s0:s1]
        slab = np.ascontiguousarray(slab.reshape(2, 128, HS, PLANE))
        in_maps.append({
            "xs": slab, "wkh": wkh, "uco": uco, "ukw": ukw,
            "ukd": ukd, "bias_t": bias_t,
        })
    return in_maps


def kernel(x, U_kh, U_kw, U_kd, U_c_in, U_c_out, bias, _trace=False):
    from concourse.bass_utils import run_bass_kernel_spmd

    if "nc" not in _cache:
        _cache["nc"] = _build_program()
    nc = _cache["nc"]

    in_maps = _host_prep(x, U_kh, U_kw, U_kd, U_c_in, U_c_out, bias)
    res = run_bass_kernel_spmd(nc, in_maps, core_ids=list(range(NCORES)),
                               trace=_trace)
    _cache["last_result"] = res

    out = np.empty((1, CO, S, S, S), np.float32)
    for c in range(NCORES):
        o = res.results[c]["out"]                        # [2, 128, HP, PLANE]
        out[0, :, c * HP:(c + 1) * HP] = o.reshape(CO, HP, S, S)
    return out

